# revision 1
# baseline (speedup 1.0000x reference)
"""BallQuery kernel for Trainium2 (Bass/Tile), data-parallel over batch on 8
cores.  Transposed-orientation redesign.

Problem: xyz (8, 16384, 3) points, new_xyz (8, 1024, 3) query centers.
For each query, return the first NSAMPLE=32 point indices (ascending) with
squared distance < RADIUS^2; pad with the first found index; all-sentinel
(N+1) rows when no point is in the ball.  Output int32 (8, 1024, 32).

Algorithm per core (one batch):
  - Points live on PE partitions in chunks of 128; queries on the free axis.
    A K=24 bf16 matmul (3-way bf16 split of coords/norms, exact to ~3e-7)
    computes s = R2 - d2 for a 128-point chunk x all 1024 queries.
  - ACT Sign(s) -> h' in {-1,+1} bf16, written [point, query] in SBUF.
  - A second tiny matmul per (chunk, query-tile) with h' as the stationary
    operand and bit weights 2^t as the moving operand produces, per group of
    16 consecutive points, psumM = 2*mask16 - 65535 with mask16 the exact
    in-ball bit pattern of the group (queries back on partitions).
  - Pool drains psum to a [128 q, 1024 group] u32 plane: mask16, then keys
    key = ((1024-j) << 16) | mask16 gated on mask16 != 0.
  - DVE top-32 via 4x max8 + 3x match_replace (u32, exact integer sort)
    selects the first 32 non-empty groups; shift/and arithmetic decodes the
    embedded masks into 512 candidate values bit*(16384 - n); a final
    max8/match_replace round extracts the true first-32 hits; baseline
    padding semantics finish the row.
"""

import os
import numpy as np

import concourse.bass as bass
import concourse.bacc as bacc
import concourse.mybir as mybir
import concourse.tile as tile
from concourse import bass_utils

F32 = mybir.dt.float32
BF16 = mybir.dt.bfloat16
I32 = mybir.dt.int32
U32 = mybir.dt.uint32

N = 16384  # points per batch
M = 1024  # queries per batch
B = 8  # batches == cores
NS = 32  # samples per query
R2 = 0.15 * 0.15
KD = 24  # distance-matmul contraction rows
G = 8  # points per group
NG = N // G  # 2048 groups per query
NQ = 8  # coarse slices per m-tile
QB = [256 * i for i in range(9)]  # slice bounds in groups
NCH = N // 128  # 128 point chunks
CB = 8  # chunks per cbatch
NCB = NCH // CB  # 16 cbatches
NMT = M // 128  # 8 query m-tiles
SENT = float(N + 1)

mul = mybir.AluOpType.mult
add = mybir.AluOpType.add
sub = mybir.AluOpType.subtract
lsr = mybir.AluOpType.logical_shift_right
lsl = mybir.AluOpType.logical_shift_left
band = mybir.AluOpType.bitwise_and
bor = mybir.AluOpType.bitwise_or
isgt = mybir.AluOpType.is_gt
iseq = mybir.AluOpType.is_equal


def build(nc: bass.Bass, repeat: int = 1):
    xs_t = nc.dram_tensor("xs", [KD, N], BF16, kind="ExternalInput")
    qm_t = nc.dram_tensor("qm", [KD, M], BF16, kind="ExternalInput")
    wg_t = nc.dram_tensor("wg", [128, 16], BF16, kind="ExternalInput")
    iotaj_t = nc.dram_tensor("iotaj", [128, NG], U32, kind="ExternalInput")
    tpat_t = nc.dram_tensor("tpat", [128, NS * G], U32, kind="ExternalInput")
    out_t = nc.dram_tensor("out", [M, NS], I32, kind="ExternalOutput")
    out_ap = out_t.ap()
    dbg = os.environ.get("BALLQ_DBG", "")
    dbg_t = None
    if dbg:
        dbg_t = nc.dram_tensor("dbg", [128, NMT * NG], U32, kind="ExternalOutput")

    with tile.TileContext(nc) as tc:
        import contextlib

        with contextlib.ExitStack() as ctx:
            const_pool = ctx.enter_context(tc.tile_pool(name="const", bufs=1))
            h_pool = ctx.enter_context(tc.tile_pool(name="h", bufs=20))
            dps_pool = ctx.enter_context(
                tc.tile_pool(name="dps", bufs=2, space="PSUM")
            )
            mps_pool = ctx.enter_context(
                tc.tile_pool(name="mps", bufs=2, space="PSUM")
            )
            gate_pool = ctx.enter_context(tc.tile_pool(name="gate", bufs=2))
            qv_pool = ctx.enter_context(tc.tile_pool(name="qv", bufs=1))
            cand_pool = ctx.enter_context(tc.tile_pool(name="cand", bufs=2))
            small_pool = ctx.enter_context(tc.tile_pool(name="small", bufs=4))

            # ---------------- constants ----------------
            xs = const_pool.tile([KD, N], BF16)
            for sl in range(8):
                w = N // 8
                nc.sync.dma_start(
                    xs[:, sl * w : (sl + 1) * w], xs_t.ap()[:, sl * w : (sl + 1) * w]
                )
            qm = const_pool.tile([KD, M], BF16)
            nc.sync.dma_start(qm[:], qm_t.ap())
            wg = const_pool.tile([128, 16], BF16)
            nc.sync.dma_start(wg[:], wg_t.ap())
            iotaj = const_pool.tile([128, NG], U32)
            nc.sync.dma_start(iotaj[:], iotaj_t.ap())
            tpat = const_pool.tile([128, NS * G], U32)
            nc.sync.dma_start(tpat[:], tpat_t.ap())
            sent = const_pool.tile([128, 1], I32)
            nc.vector.memset(sent[:], SENT)
            bias_half = const_pool.tile([128, 1], F32)
            nc.vector.memset(bias_half[:], 127.5)
            bias_n = const_pool.tile([128, 1], F32)
            nc.vector.memset(bias_n[:], float(N))
            bias_zero = const_pool.tile([128, 1], F32)
            nc.vector.memset(bias_zero[:], 0.0)
            bias_zero = const_pool.tile([128, 1], F32)
            nc.vector.memset(bias_zero[:], 0.0)

            # mask/key plane: [128 q, mt, j] u32, one col block per group
            maskpl = const_pool.tile([128, NMT * NG], U32)
            mp3 = maskpl[:].rearrange("p (t j) -> p t j", t=NMT)

            for rep in range(repeat):
                hts = [None] * NCH

                def dist_cbatch(cb):
                    for ci in range(CB):
                        c = cb * CB + ci
                        dps = dps_pool.tile([128, M], F32)
                        for half in range(2):
                            nc.tensor.matmul(
                                dps[:, half * 512 : (half + 1) * 512],
                                xs[:, c * 128 : (c + 1) * 128],
                                qm[:, half * 512 : (half + 1) * 512],
                                start=True,
                                stop=True,
                            )
                        h = h_pool.tile([128, M], BF16)
                        nc.scalar.activation(
                            h[:], dps[:], mybir.ActivationFunctionType.Sign
                        )
                        hts[c] = h

                def mask_cbatch(cb):
                    gpc = 128 // G  # 16 groups per chunk
                    mps = mps_pool.tile([128, NMT * CB * gpc], F32)
                    for mt in range(NMT):
                        for ci in range(CB):
                            c = cb * CB + ci
                            o = mt * CB * gpc + ci * gpc
                            nc.tensor.matmul(
                                mps[:, o : o + gpc],
                                hts[c][:, mt * 128 : (mt + 1) * 128],
                                wg[:, 0:gpc],
                                start=True,
                                stop=True,
                            )
                    # drain psum -> mask plane: mask8 = psum*0.5 + 127.5
                    nc.scalar.activation(
                        mp3[:, :, cb * 128 : (cb + 1) * 128],
                        mps[:].rearrange("p (t w) -> p t w", t=NMT),
                        mybir.ActivationFunctionType.Identity,
                        bias=bias_half[:],
                        scale=0.5,
                    )

                skips = set(os.environ.get("BALLQ_SKIP", "").split(","))
                qvs = [
                    qv_pool.tile([128, NQ * NS], U32, name=f"qv{i}")
                    for i in range(NMT)
                ]

                def quarter(q):
                    lo, hi = QB[q], QB[q + 1]
                    for mt in range(NMT):
                        key = mp3[:, mt, lo:hi]
                        gate = gate_pool.tile([128, hi - lo], U32)
                        nc.gpsimd.tensor_scalar(gate[:], key, 0.0, None, op0=isgt)
                        nc.gpsimd.tensor_tensor(
                            gate[:], gate[:], iotaj[:, lo:hi], mul
                        )
                        nc.gpsimd.tensor_tensor(key, key, gate[:], add)
                        qv = qvs[mt]
                        for r in range(4):
                            nc.vector.max(
                                qv[:, q * NS + 8 * r : q * NS + 8 * r + 8], key
                            )
                            if r < 3:
                                nc.vector.match_replace(
                                    out=key,
                                    in_to_replace=qv[
                                        :, q * NS + 8 * r : q * NS + 8 * r + 8
                                    ],
                                    in_values=key,
                                    imm_value=0.0,
                                )

                qbound = {2 * (i + 1): i for i in range(NQ - 1)}
                dist_cbatch(0)
                for cb in range(1, NCB):
                    mask_cbatch(cb - 1)
                    dist_cbatch(cb)
                    if cb in qbound:
                        quarter(qbound[cb])
                mask_cbatch(NCB - 1)
                quarter(NQ - 1)
                if dbg == "mask":
                    nc.sync.dma_start(dbg_t.ap(), maskpl[:])

                # ---------------- per m-tile merge + decode ----------------
                for mt in range(NMT):
                    qv = qvs[mt]
                    vals = small_pool.tile([128, NS], U32)
                    for r in range(4):
                        nc.vector.max(vals[:, 8 * r : 8 * r + 8], qv[:])
                        if r < 3:
                            nc.vector.match_replace(
                                out=qv[:],
                                in_to_replace=vals[:, 8 * r : 8 * r + 8],
                                in_values=qv[:],
                                imm_value=0.0,
                            )
                    if dbg == "vals":
                        nc.sync.dma_start(
                            dbg_t.ap()[:, mt * NS : (mt + 1) * NS], vals[:]
                        )

                    # decode: A = (vals >> 8) << 3 ; msk = vals & 0xff
                    A = small_pool.tile([128, NS], U32)
                    msk = small_pool.tile([128, NS], U32)
                    nc.vector.tensor_scalar(A[:], vals[:], 8, 3, op0=lsr, op1=lsl)
                    nc.vector.tensor_scalar(msk[:], vals[:], 255, None, op0=band)

                    cw = NS * G  # 256 candidates
                    bits = cand_pool.tile([128, cw], U32)
                    cand = cand_pool.tile([128, cw], U32)
                    b3 = bits[:].rearrange("p (i t) -> p i t", i=NS)
                    c3 = cand[:].rearrange("p (i t) -> p i t", i=NS)
                    t3 = tpat[:].rearrange("p (i t) -> p i t", i=NS)
                    mskb = (
                        msk[:].rearrange("p (i o) -> p i o", o=1)
                        .to_broadcast([128, NS, G])
                    )
                    Ab = (
                        A[:].rearrange("p (i o) -> p i o", o=1)
                        .to_broadcast([128, NS, G])
                    )
                    # bits = (msk >> t) & 1 ; cand = bits * (A - t)
                    nc.vector.tensor_tensor(b3, mskb, t3, lsr)
                    nc.vector.tensor_scalar(bits[:], bits[:], 1, None, op0=band)
                    nc.vector.tensor_tensor(c3, Ab, t3, sub)
                    nc.vector.tensor_tensor(cand[:], cand[:], bits[:], mul)
                    if dbg == "cand":
                        nc.sync.dma_start(
                            dbg_t.ap()[:, mt * cw : (mt + 1) * cw], cand[:]
                        )

                    fv = small_pool.tile([128, NS], U32)
                    for r in range(4):
                        nc.vector.max(fv[:, 8 * r : 8 * r + 8], cand[:])
                        if r < 3:
                            nc.vector.match_replace(
                                out=cand[:],
                                in_to_replace=fv[:, 8 * r : 8 * r + 8],
                                in_values=cand[:],
                                imm_value=0.0,
                            )

                    # idx = 16384 - v with padding semantics
                    idxf = small_pool.tile([128, NS], I32)
                    nc.vector.tensor_scalar(
                        idxf[:], fv[:], -1.0, float(N), op0=mul, op1=add
                    )
                    inv = small_pool.tile([128, NS], U32)
                    nc.vector.tensor_scalar(inv[:], fv[:], 0.0, None, op0=iseq)
                    nc.vector.copy_predicated(
                        idxf[:], inv[:], idxf[:, 0:1].to_broadcast([128, NS])
                    )
                    nc.vector.copy_predicated(
                        idxf[:],
                        inv[:, 0:1].to_broadcast([128, NS]),
                        sent[:].to_broadcast([128, NS]),
                    )
                    nc.sync.dma_start(
                        out_ap[mt * 128 : (mt + 1) * 128, :], idxf[:]
                    )

    return nc


def _split3(v):
    """3-way bf16 split of float64 array v: v ~ s0 + s1 + s2 (each bf16)."""
    import ml_dtypes

    s0 = v.astype(ml_dtypes.bfloat16)
    r1 = v - s0.astype(np.float64)
    s1 = r1.astype(ml_dtypes.bfloat16)
    r2 = r1 - s1.astype(np.float64)
    s2 = r2.astype(ml_dtypes.bfloat16)
    return s0, s1, s2


def host_prep(xyz_b: np.ndarray, q_b: np.ndarray) -> dict:
    """Per-batch input prep: build xs/qm split tensors + constants."""
    import ml_dtypes

    BF = ml_dtypes.bfloat16
    x = xyz_b.astype(np.float64)  # [N, 3]
    q = q_b.astype(np.float64)  # [M, 3]
    x0, x1, x2 = _split3(x)
    q0, q1, q2 = _split3(q)
    # C_m = R2 - |q|^2 with |q|^2 in f32 to mirror the reference's rounding
    qsq = np.sum(q_b.astype(np.float32) * q_b.astype(np.float32), axis=1)
    C = np.float64(R2) - qsq.astype(np.float64)
    c0, c1, c2 = _split3(C)
    xsq = np.sum(xyz_b.astype(np.float32) * xyz_b.astype(np.float32), axis=1)
    D = -xsq.astype(np.float64)
    d0, d1, d2 = _split3(D)

    ones_n = np.ones(N, BF)
    ones_m = np.ones(M, BF)
    xs_rows = [ones_n, ones_n, ones_n, d0, d1, d2]
    qm_rows = [c0, c1, c2, ones_m, ones_m, ones_m]
    pairs = [(x0, q0), (x0, q1), (x1, q0), (x0, q2), (x2, q0), (x1, q1)]
    for xsplit, qsplit in pairs:
        two_x = (2.0 * xsplit.astype(np.float64)).astype(BF)
        for d in range(3):
            xs_rows.append(two_x[:, d])
            qm_rows.append(qsplit[:, d])
    xs = np.stack(xs_rows, axis=0)  # [24, N] bf16
    qmv = np.stack(qm_rows, axis=0)  # [24, M] bf16

    wg = np.zeros((128, 128 // G), BF)
    for p in range(128):
        wg[p, p // G] = float(2 ** (p % G))

    iotaj = np.broadcast_to(
        ((NG - np.arange(NG, dtype=np.uint32)).astype(np.uint32) << 8)[None, :],
        (128, NG),
    ).copy()
    tpat = np.broadcast_to(
        np.tile(np.arange(G, dtype=np.uint32), NS)[None, :], (128, NS * G)
    ).copy()
    return {"xs": xs, "qm": qmv, "wg": wg, "iotaj": iotaj, "tpat": tpat}


_NC_CACHE = {}
LAST_RESULT = None
TRACE = bool(int(os.environ.get("BALLQ_TRACE", "0")))


def _get_nc(repeat: int = 1):
    if repeat not in _NC_CACHE:
        nc = bacc.Bacc("TRN2", target_bir_lowering=False, debug=False)
        build(nc, repeat)
        nc.compile()
        _NC_CACHE[repeat] = nc
    return _NC_CACHE[repeat]


def kernel(**inputs) -> np.ndarray:
    global LAST_RESULT
    xyz = np.ascontiguousarray(np.asarray(inputs["xyz"], dtype=np.float32))
    new_xyz = np.ascontiguousarray(np.asarray(inputs["new_xyz"], dtype=np.float32))
    assert xyz.shape == (B, N, 3) and new_xyz.shape == (B, M, 3)

    nc = _get_nc(int(os.environ.get("BALLQ_REPEAT", "1")))
    in_maps = [host_prep(xyz[b], new_xyz[b]) for b in range(B)]
    res = bass_utils.run_bass_kernel_spmd(nc, in_maps, list(range(B)), trace=TRACE)
    LAST_RESULT = res
    out = np.stack([res.results[b]["out"] for b in range(B)], axis=0)
    return out.astype(np.int32)

